# revision 24
# baseline (speedup 1.0000x reference)
"""Trainium2 kernel for nn_ATThippomod (B=32,T=256,D=256,H=512,TEVAL=32).

Device (8 NeuronCores, data-parallel over batch, 4 samples/core): batched
Newton-Schulz inversion of A=Z^T with a tuned scaling schedule - the dominant
compute (~100 GFlop of 256^3 matmuls). Transpose-free V/W pairing:
    P  = A @ V          (lhsT = Z natural layout)
    E  = a_k*I - P
    V' = V @ E          (lhsT = W)
    W' = E^T @ W        (lhsT = E)
V converges to A^{-1} = Z^{-T}, which equals the reference's
(U/S)@Vh pseudo-inverse since Z is square/invertible.
Host: GRU scan, euler ODE loop (refactored so datt_dz is never materialized),
conv output head.
"""
import numpy as np

B, TSEQ, DIN, DOUT, HID, TEVAL = 32, 256, 128, 256, 512, 32
DELTA_T = np.float32(0.01)
NCORES = 8
BPC = B // NCORES
SCHED = [3.0] * 26 + [2.8] + [2.0] * 8
NIT = len(SCHED)

_CACHE = {}


def _build_ns_bass():
    import concourse.bass as bass
    import concourse.mybir as mybir
    from concourse.tile import TileContext

    f32 = mybir.dt.float32
    nc = bass.Bass()
    zvw_in = nc.declare_dram_parameter("zvw", [3 * BPC + 1, 256, 256], f32, isOutput=False)
    v_out = nc.declare_dram_parameter("vout", [BPC, 256, 256], f32, isOutput=True)

    with TileContext(nc) as tc:
        with (
            tc.tile_pool(name="resid", bufs=1) as rp,
            tc.tile_pool(name="work", bufs=3) as wk,
            tc.tile_pool(name="ps", bufs=2, space="PSUM") as ps,
        ):
            # DMA -> staging, then DVE copy -> working tile: every matmul
            # operand has exactly ONE writer (VectorE), so walrus's fused
            # LDWEIGHTS struct never carries more than one sync wait.

            zvw_st = rp.tile([128, (3 * BPC + 1) * 512], f32, tag="zvw_st")
            nc.gpsimd.dma_start(
                out=zvw_st[:, :].rearrange("p (n r d) -> p n r d", n=3 * BPC + 1, r=2),
                in_=zvw_in.rearrange("n (r p) d -> p n r d", p=128))
            # identity I128 rides along as the last packed matrix (r=0 block)
            ident = zvw_st[:, 3 * BPC * 512: 3 * BPC * 512 + 128]
            zt, vcur, wcur, vnxt, wnxt = [], [], [], [], []
            for s in range(BPC):
                z_s = rp.tile([128, 512], f32, tag=f"z{s}")
                v_s = rp.tile([128, 512], f32, tag=f"v{s}")
                w_s = rp.tile([128, 512], f32, tag=f"w{s}")
                for ci, dst in ((0, z_s), (1, v_s), (2, w_s)):
                    n = s * 3 + ci
                    nc.vector.tensor_copy(dst[:, :], zvw_st[:, n * 512:(n + 1) * 512])
                zt.append(z_s); vcur.append(v_s); wcur.append(w_s)
                vb_s = rp.tile([128, 512], f32, tag=f"vb{s}")
                wb_s = rp.tile([128, 512], f32, tag=f"wb{s}")
                vnxt.append(vb_s); wnxt.append(wb_s)

            for it, _a in enumerate(SCHED):
                aslc = wk.tile([128, 128], f32, tag="aik")
                nc.vector.tensor_scalar(out=aslc[:, :], in0=ident, scalar1=float(_a),
                                        scalar2=None, op0=mybir.AluOpType.mult)
                aslc = aslc[:, :]
                for s in range(BPC):
                    z_s, v_s, w_s = zt[s], vcur[s], wcur[s]
                    v_n, w_n = vnxt[s], wnxt[s]
                    e_s = wk.tile([128, 512], f32, tag=f"e{s}")
                    # ---- P = A @ V,  E = a*I - P ----
                    for m in range(2):
                        p_ps = ps.tile([128, 256], f32, tag="pp")
                        for k in range(2):
                            nc.tensor.matmul(
                                p_ps[:, :],
                                z_s[:, k * 256 + m * 128: k * 256 + m * 128 + 128],
                                v_s[:, k * 256:(k + 1) * 256],
                                start=(k == 0), stop=(k == 1),
                            )
                        # E m-block = a*I - P, built entirely on VectorE:
                        # off-diagonal half: -P ; diagonal half: aI - P
                        od = (1 - m)
                        nc.vector.tensor_scalar(
                            out=e_s[:, m * 256 + od * 128: m * 256 + od * 128 + 128],
                            in0=p_ps[:, od * 128:od * 128 + 128],
                            scalar1=-1.0, scalar2=None, op0=mybir.AluOpType.mult,
                        )
                        nc.vector.tensor_tensor(
                            out=e_s[:, m * 256 + m * 128: m * 256 + m * 128 + 128],
                            in0=aslc, in1=p_ps[:, m * 128:m * 128 + 128],
                            op=mybir.AluOpType.subtract,
                        )
                    # ---- V' = V @ E (lhsT = W) ; W' = E^T @ W (lhsT = E) ----
                    for m in range(2):
                        vp_ps = ps.tile([128, 256], f32, tag="vo")
                        wp_ps = ps.tile([128, 256], f32, tag="wo")
                        for k in range(2):
                            nc.tensor.matmul(
                                vp_ps[:, :],
                                w_s[:, k * 256 + m * 128: k * 256 + m * 128 + 128],
                                e_s[:, k * 256:(k + 1) * 256],
                                start=(k == 0), stop=(k == 1),
                            )
                        for k in range(2):
                            nc.tensor.matmul(
                                wp_ps[:, :],
                                e_s[:, k * 256 + m * 128: k * 256 + m * 128 + 128],
                                w_s[:, k * 256:(k + 1) * 256],
                                start=(k == 0), stop=(k == 1),
                            )
                        nc.vector.tensor_copy(v_n[:, m * 256:(m + 1) * 256], vp_ps[:, :])
                        nc.vector.tensor_copy(w_n[:, m * 256:(m + 1) * 256], wp_ps[:, :])
                if True:
                    vcur, vnxt = vnxt, vcur
                    wcur, wnxt = wnxt, wcur

            vfin = rp.tile([128, BPC * 512], f32, tag="vfin")
            for s in range(BPC):
                nc.vector.tensor_copy(vfin[:, s * 512:(s + 1) * 512], vcur[s][:, :])
            out_dma = nc.gpsimd.dma_start(
                out=v_out.rearrange("s (r p) d -> p s r d", p=128),
                in_=vfin[:, :].rearrange("p (s r d) -> p s r d", s=BPC, r=2),
            )

    # --- workaround: this walrus build rejects >1 sync-wait per instruction.
    # The kernel-tail Drain aggregates {PE, DVE, input-queue, output-queue}
    # waits, but all except the output-DMA queue are transitively covered
    # (PE -> DVE copies -> gather copies -> output DMA's own DVE wait; the
    # input DMA was consumed by DVE at the start), and the all-engine barrier
    # right after the drain handles engine quiescence. Keep only the
    # output-queue wait.
    upd = out_dma.ins.sync_info.on_update if hasattr(out_dma, "ins") else None
    out_sem_ids = {u.id for u in (upd or [])}
    for inst in nc.all_instructions():
        si = getattr(inst, "sync_info", None)
        if si is None or not si.on_wait or len(si.on_wait) <= 1:
            continue
        if inst.opcode_name() if hasattr(inst, "opcode_name") else True:
            pass
        keep = [w for w in si.on_wait if w.id in out_sem_ids]
        if not keep:
            keep = [w for w in si.on_wait if "DMA" in getattr(w, "ant_name", "")][-1:]
        if not keep:
            keep = list(si.on_wait)[-1:]
        si.on_wait = keep
    return nc


def _device_inverse(Z, want_trace=False):
    """Z: (B,256,256) f32 -> Z^{-T} via on-device Newton-Schulz. Returns (Zpinv, exec_ns)."""
    from concourse.bass_utils import run_bass_kernel_spmd

    if "nc" not in _CACHE:
        _CACHE["nc"] = _build_ns_bass()
    nc = _CACHE["nc"]

    fro2 = (Z.astype(np.float64) ** 2).sum((1, 2)).astype(np.float32)
    c = (np.float32(2.99) / fro2).astype(np.float32)[:, None, None]
    V0 = (c * Z).astype(np.float32)
    W0 = np.ascontiguousarray(np.swapaxes(V0, 1, 2))

    in_maps = []
    for i in range(NCORES):
        sl = slice(i * BPC, (i + 1) * BPC)
        zvw = np.stack([Z[sl], V0[sl], W0[sl]], axis=1).reshape(3 * BPC, 256, 256)
        zvw = np.concatenate([zvw, np.eye(256, dtype=np.float32)[None]], axis=0)
        in_maps.append({"zvw": np.ascontiguousarray(zvw)})
    res = run_bass_kernel_spmd(nc, in_maps, list(range(NCORES)), trace=want_trace)
    out = np.concatenate([res.results[i]["vout"] for i in range(NCORES)], axis=0)
    return out, getattr(res, "exec_time_ns", None)


def kernel(**inputs):
    inputs = {k: np.asarray(v) for k, v in inputs.items()}
    times = inputs["times"].astype(np.float32)
    Y = inputs["Y"].astype(np.float32)
    W_ih, W_hh = inputs["W_ih"].astype(np.float32), inputs["W_hh"].astype(np.float32)
    b_ih, b_hh = inputs["b_ih"].astype(np.float32), inputs["b_hh"].astype(np.float32)

    # ---------------- host: GRU scan ----------------
    def sig(x):
        return (1.0 / (1.0 + np.exp(-x))).astype(np.float32)

    h = inputs["gru_h0"].astype(np.float32).copy()
    Z = np.empty((B, TSEQ, DOUT), np.float32)
    GI = (Y.reshape(-1, DIN) @ W_ih.T + b_ih).reshape(B, TSEQ, 3 * DOUT).astype(np.float32)
    for t in range(TSEQ):
        gh = (h @ W_hh.T + b_hh).astype(np.float32)
        i_r, i_z, i_n = np.split(GI[:, t], 3, -1)
        h_r, h_z, h_n = np.split(gh, 3, -1)
        r = sig(i_r + h_r)
        z = sig(i_z + h_z)
        n = np.tanh(i_n + r * h_n).astype(np.float32)
        h = ((1.0 - z) * n + z * h).astype(np.float32)
        Z[:, t] = h

    # ------------- device: Z_pinv = Z^{-T} (+ Wv column scaling; Wv=ones) ----
    Zpinv, _ = _device_inverse(Z)
    Zpinv = (Zpinv * inputs["Wv"][np.newaxis, 0:1, :]).astype(np.float32)

    # ---------------- host: euler ODE loop ----------------
    freqs64 = np.exp2(np.arange(DOUT // 2, dtype=np.float64))
    embed = np.empty((B, TEVAL, DOUT), np.float32)
    y = inputs["init_embed"].astype(np.float32).copy()
    embed[:, 0] = y
    Wz1, bz1 = inputs["Wz1"], inputs["bz1"]
    Wz2, bz2 = inputs["Wz2"], inputs["bz2"]
    Wz3, bz3 = inputs["Wz3"], inputs["bz3"]
    for k in range(TEVAL - 1):
        tv = np.float64(times[k])
        enc = np.stack([np.sin(freqs64 * tv), np.cos(freqs64 * tv)], -1).reshape(-1).astype(np.float32)
        att1 = y + enc
        p = np.einsum('btd,bd->bt', Zpinv, att1)
        p = p - p.min(1, keepdims=True)
        p = (p / p.sum(1, keepdims=True)).astype(np.float32)
        zp = np.einsum('btd,bt->bd', Z, p)
        zt = np.einsum('bt,btd->bd', p, Zpinv)
        h1 = np.maximum(zt @ Wz1.T + bz1, 0).astype(np.float32)
        h2 = np.maximum(h1 @ Wz2.T + bz2, 0).astype(np.float32)
        dz = (h2 @ Wz3.T + bz3).astype(np.float32)
        w = p * np.einsum('bte,be->bt', Z, dz)
        out = np.einsum('bt,bte->be', w, Z) - (dz * zp).sum(1, keepdims=True) * zp
        y = (y + DELTA_T * out).astype(np.float32)
        embed[:, k + 1] = y

    # ---------------- host: output head ----------------
    Wo1, bo1, Wo3, bo3 = inputs["Wo1"], inputs["bo1"], inputs["Wo3"], inputs["bo3"]
    pred = np.maximum(embed @ Wo1.T + bo1, 0).astype(np.float32)
    pred = np.swapaxes(pred, 1, 2)  # (B,H,T)
    for Wc, bc in ((inputs["Wc1"], inputs["bc1"]), (inputs["Wc2"], inputs["bc2"])):
        xp = np.pad(pred, ((0, 0), (0, 0), (1, 1)))
        conv = (np.einsum('bit,oi->bot', xp[:, :, 0:TEVAL], Wc[:, :, 0])
                + np.einsum('bit,oi->bot', xp[:, :, 1:TEVAL + 1], Wc[:, :, 1])
                + np.einsum('bit,oi->bot', xp[:, :, 2:TEVAL + 2], Wc[:, :, 2])
                + bc[None, :, None]).astype(np.float32)
        pred = (np.maximum(conv, 0) + pred).astype(np.float32)
    pred = np.swapaxes(pred, 1, 2)
    y_traj = (pred @ Wo3.T + bo3).astype(np.float32)
    embed_flat = embed.reshape(B, -1)
    return (y_traj, y_traj, times, embed_flat)


# revision 25
# speedup vs baseline: 1.0761x; 1.0761x over previous
"""Trainium2 kernel for nn_ATThippomod (B=32,T=256,D=256,H=512,TEVAL=32).

Device (8 NeuronCores, data-parallel over batch, 4 samples/core): batched
Newton-Schulz inversion of A=Z^T with a tuned scaling schedule - the dominant
compute (~100 GFlop of 256^3 matmuls). Transpose-free V/W pairing:
    P  = A @ V          (lhsT = Z natural layout)
    E  = a_k*I - P
    V' = V @ E          (lhsT = W)
    W' = E^T @ W        (lhsT = E)
V converges to A^{-1} = Z^{-T}, which equals the reference's
(U/S)@Vh pseudo-inverse since Z is square/invertible.
Host: GRU scan, euler ODE loop (refactored so datt_dz is never materialized),
conv output head.
"""
import numpy as np

B, TSEQ, DIN, DOUT, HID, TEVAL = 32, 256, 128, 256, 512, 32
DELTA_T = np.float32(0.01)
NCORES = 8
BPC = B // NCORES
SCHED = [3.0] * 26 + [2.8] + [2.0] * 8
NIT = len(SCHED)

_CACHE = {}


def _build_ns_bass():
    import concourse.bass as bass
    import concourse.mybir as mybir
    from concourse.tile import TileContext

    f32 = mybir.dt.float32
    nc = bass.Bass()
    zvw_in = nc.declare_dram_parameter("zvw", [3 * BPC + 1, 256, 256], f32, isOutput=False)
    v_out = nc.declare_dram_parameter("vout", [BPC, 256, 256], f32, isOutput=True)

    with TileContext(nc) as tc:
        with (
            tc.tile_pool(name="resid", bufs=1) as rp,
            tc.tile_pool(name="work", bufs=3) as wk,
            tc.tile_pool(name="ps", bufs=2, space="PSUM") as ps,
        ):
            # DMA -> staging, then DVE copy -> working tile: every matmul
            # operand has exactly ONE writer (VectorE), so walrus's fused
            # LDWEIGHTS struct never carries more than one sync wait.

            zvw_st = rp.tile([128, (3 * BPC + 1) * 512], f32, tag="zvw_st")
            nc.gpsimd.dma_start(
                out=zvw_st[:, :].rearrange("p (n r d) -> p n r d", n=3 * BPC + 1, r=2),
                in_=zvw_in.rearrange("n (r p) d -> p n r d", p=128))
            # identity I128 rides along as the last packed matrix (r=0 block)
            ident = zvw_st[:, 3 * BPC * 512: 3 * BPC * 512 + 128]
            zt, vcur, wcur, vnxt, wnxt = [], [], [], [], []
            for s in range(BPC):
                z_s = rp.tile([128, 512], f32, tag=f"z{s}")
                v_s = rp.tile([128, 512], f32, tag=f"v{s}")
                w_s = rp.tile([128, 512], f32, tag=f"w{s}")
                for ci, dst in ((0, z_s), (1, v_s), (2, w_s)):
                    n = s * 3 + ci
                    nc.vector.tensor_copy(dst[:, :], zvw_st[:, n * 512:(n + 1) * 512])
                zt.append(z_s); vcur.append(v_s); wcur.append(w_s)
                vb_s = rp.tile([128, 512], f32, tag=f"vb{s}")
                wb_s = rp.tile([128, 512], f32, tag=f"wb{s}")
                vnxt.append(vb_s); wnxt.append(wb_s)

            vfin = rp.tile([128, BPC * 512], f32, tag="vfin")
            for it, _a in enumerate(SCHED):
                aslc = wk.tile([128, 128], f32, tag="aik")
                nc.vector.tensor_scalar(out=aslc[:, :], in0=ident, scalar1=float(_a),
                                        scalar2=None, op0=mybir.AluOpType.mult)
                aslc = aslc[:, :]
                for s in range(BPC):
                    z_s, v_s, w_s = zt[s], vcur[s], wcur[s]
                    v_n, w_n = vnxt[s], wnxt[s]
                    e_s = wk.tile([128, 512], f32, tag=f"e{s}")
                    # ---- P = A @ V,  E = a*I - P ----
                    for m in range(2):
                        p_ps = ps.tile([128, 256], f32, tag="pp")
                        for k in range(2):
                            nc.tensor.matmul(
                                p_ps[:, :],
                                z_s[:, k * 256 + m * 128: k * 256 + m * 128 + 128],
                                v_s[:, k * 256:(k + 1) * 256],
                                start=(k == 0), stop=(k == 1),
                            )
                        # E m-block = a*I - P, built entirely on VectorE:
                        # off-diagonal half: -P ; diagonal half: aI - P
                        od = (1 - m)
                        nc.vector.tensor_scalar(
                            out=e_s[:, m * 256 + od * 128: m * 256 + od * 128 + 128],
                            in0=p_ps[:, od * 128:od * 128 + 128],
                            scalar1=-1.0, scalar2=None, op0=mybir.AluOpType.mult,
                        )
                        nc.vector.tensor_tensor(
                            out=e_s[:, m * 256 + m * 128: m * 256 + m * 128 + 128],
                            in0=aslc, in1=p_ps[:, m * 128:m * 128 + 128],
                            op=mybir.AluOpType.subtract,
                        )
                    # ---- V' = V @ E (lhsT = W) ; W' = E^T @ W (lhsT = E) ----
                    last = (it == NIT - 1)
                    for m in range(2):
                        vp_ps = ps.tile([128, 256], f32, tag="vo")
                        for k in range(2):
                            nc.tensor.matmul(
                                vp_ps[:, :],
                                w_s[:, k * 256 + m * 128: k * 256 + m * 128 + 128],
                                e_s[:, k * 256:(k + 1) * 256],
                                start=(k == 0), stop=(k == 1),
                            )
                        if last:
                            # final V' goes straight into the output-gather tile;
                            # W' of the last iteration is never consumed - skip it.
                            nc.vector.tensor_copy(
                                vfin[:, s * 512 + m * 256: s * 512 + (m + 1) * 256],
                                vp_ps[:, :])
                            continue
                        wp_ps = ps.tile([128, 256], f32, tag="wo")
                        for k in range(2):
                            nc.tensor.matmul(
                                wp_ps[:, :],
                                e_s[:, k * 256 + m * 128: k * 256 + m * 128 + 128],
                                w_s[:, k * 256:(k + 1) * 256],
                                start=(k == 0), stop=(k == 1),
                            )
                        nc.vector.tensor_copy(v_n[:, m * 256:(m + 1) * 256], vp_ps[:, :])
                        nc.vector.tensor_copy(w_n[:, m * 256:(m + 1) * 256], wp_ps[:, :])
                if True:
                    vcur, vnxt = vnxt, vcur
                    wcur, wnxt = wnxt, wcur

            out_dma = nc.gpsimd.dma_start(
                out=v_out.rearrange("s (r p) d -> p s r d", p=128),
                in_=vfin[:, :].rearrange("p (s r d) -> p s r d", s=BPC, r=2),
            )

    # --- workaround: this walrus build rejects >1 sync-wait per instruction.
    # The kernel-tail Drain aggregates {PE, DVE, input-queue, output-queue}
    # waits, but all except the output-DMA queue are transitively covered
    # (PE -> DVE copies -> gather copies -> output DMA's own DVE wait; the
    # input DMA was consumed by DVE at the start), and the all-engine barrier
    # right after the drain handles engine quiescence. Keep only the
    # output-queue wait.
    upd = out_dma.ins.sync_info.on_update if hasattr(out_dma, "ins") else None
    out_sem_ids = {u.id for u in (upd or [])}
    for inst in nc.all_instructions():
        si = getattr(inst, "sync_info", None)
        if si is None or not si.on_wait or len(si.on_wait) <= 1:
            continue
        if inst.opcode_name() if hasattr(inst, "opcode_name") else True:
            pass
        keep = [w for w in si.on_wait if w.id in out_sem_ids]
        if not keep:
            keep = [w for w in si.on_wait if "DMA" in getattr(w, "ant_name", "")][-1:]
        if not keep:
            keep = list(si.on_wait)[-1:]
        si.on_wait = keep
    return nc


def _device_inverse(Z, want_trace=False):
    """Z: (B,256,256) f32 -> Z^{-T} via on-device Newton-Schulz. Returns (Zpinv, exec_ns)."""
    from concourse.bass_utils import run_bass_kernel_spmd

    if "nc" not in _CACHE:
        _CACHE["nc"] = _build_ns_bass()
    nc = _CACHE["nc"]

    fro2 = (Z.astype(np.float64) ** 2).sum((1, 2)).astype(np.float32)
    c = (np.float32(2.99) / fro2).astype(np.float32)[:, None, None]
    V0 = (c * Z).astype(np.float32)
    W0 = np.ascontiguousarray(np.swapaxes(V0, 1, 2))

    in_maps = []
    for i in range(NCORES):
        sl = slice(i * BPC, (i + 1) * BPC)
        zvw = np.stack([Z[sl], V0[sl], W0[sl]], axis=1).reshape(3 * BPC, 256, 256)
        zvw = np.concatenate([zvw, np.eye(256, dtype=np.float32)[None]], axis=0)
        in_maps.append({"zvw": np.ascontiguousarray(zvw)})
    res = run_bass_kernel_spmd(nc, in_maps, list(range(NCORES)), trace=want_trace)
    out = np.concatenate([res.results[i]["vout"] for i in range(NCORES)], axis=0)
    return out, getattr(res, "exec_time_ns", None)


def kernel(**inputs):
    inputs = {k: np.asarray(v) for k, v in inputs.items()}
    times = inputs["times"].astype(np.float32)
    Y = inputs["Y"].astype(np.float32)
    W_ih, W_hh = inputs["W_ih"].astype(np.float32), inputs["W_hh"].astype(np.float32)
    b_ih, b_hh = inputs["b_ih"].astype(np.float32), inputs["b_hh"].astype(np.float32)

    # ---------------- host: GRU scan ----------------
    def sig(x):
        return (1.0 / (1.0 + np.exp(-x))).astype(np.float32)

    h = inputs["gru_h0"].astype(np.float32).copy()
    Z = np.empty((B, TSEQ, DOUT), np.float32)
    GI = (Y.reshape(-1, DIN) @ W_ih.T + b_ih).reshape(B, TSEQ, 3 * DOUT).astype(np.float32)
    for t in range(TSEQ):
        gh = (h @ W_hh.T + b_hh).astype(np.float32)
        i_r, i_z, i_n = np.split(GI[:, t], 3, -1)
        h_r, h_z, h_n = np.split(gh, 3, -1)
        r = sig(i_r + h_r)
        z = sig(i_z + h_z)
        n = np.tanh(i_n + r * h_n).astype(np.float32)
        h = ((1.0 - z) * n + z * h).astype(np.float32)
        Z[:, t] = h

    # ------------- device: Z_pinv = Z^{-T} (+ Wv column scaling; Wv=ones) ----
    Zpinv, _ = _device_inverse(Z)
    Zpinv = (Zpinv * inputs["Wv"][np.newaxis, 0:1, :]).astype(np.float32)

    # ---------------- host: euler ODE loop ----------------
    freqs64 = np.exp2(np.arange(DOUT // 2, dtype=np.float64))
    embed = np.empty((B, TEVAL, DOUT), np.float32)
    y = inputs["init_embed"].astype(np.float32).copy()
    embed[:, 0] = y
    Wz1, bz1 = inputs["Wz1"], inputs["bz1"]
    Wz2, bz2 = inputs["Wz2"], inputs["bz2"]
    Wz3, bz3 = inputs["Wz3"], inputs["bz3"]
    for k in range(TEVAL - 1):
        tv = np.float64(times[k])
        enc = np.stack([np.sin(freqs64 * tv), np.cos(freqs64 * tv)], -1).reshape(-1).astype(np.float32)
        att1 = y + enc
        p = np.einsum('btd,bd->bt', Zpinv, att1)
        p = p - p.min(1, keepdims=True)
        p = (p / p.sum(1, keepdims=True)).astype(np.float32)
        zp = np.einsum('btd,bt->bd', Z, p)
        zt = np.einsum('bt,btd->bd', p, Zpinv)
        h1 = np.maximum(zt @ Wz1.T + bz1, 0).astype(np.float32)
        h2 = np.maximum(h1 @ Wz2.T + bz2, 0).astype(np.float32)
        dz = (h2 @ Wz3.T + bz3).astype(np.float32)
        w = p * np.einsum('bte,be->bt', Z, dz)
        out = np.einsum('bt,bte->be', w, Z) - (dz * zp).sum(1, keepdims=True) * zp
        y = (y + DELTA_T * out).astype(np.float32)
        embed[:, k + 1] = y

    # ---------------- host: output head ----------------
    Wo1, bo1, Wo3, bo3 = inputs["Wo1"], inputs["bo1"], inputs["Wo3"], inputs["bo3"]
    pred = np.maximum(embed @ Wo1.T + bo1, 0).astype(np.float32)
    pred = np.swapaxes(pred, 1, 2)  # (B,H,T)
    for Wc, bc in ((inputs["Wc1"], inputs["bc1"]), (inputs["Wc2"], inputs["bc2"])):
        xp = np.pad(pred, ((0, 0), (0, 0), (1, 1)))
        conv = (np.einsum('bit,oi->bot', xp[:, :, 0:TEVAL], Wc[:, :, 0])
                + np.einsum('bit,oi->bot', xp[:, :, 1:TEVAL + 1], Wc[:, :, 1])
                + np.einsum('bit,oi->bot', xp[:, :, 2:TEVAL + 2], Wc[:, :, 2])
                + bc[None, :, None]).astype(np.float32)
        pred = (np.maximum(conv, 0) + pred).astype(np.float32)
    pred = np.swapaxes(pred, 1, 2)
    y_traj = (pred @ Wo3.T + bo3).astype(np.float32)
    embed_flat = embed.reshape(B, -1)
    return (y_traj, y_traj, times, embed_flat)
